# revision 27
# baseline (speedup 1.0000x reference)
"""Causal self-attention (B=2, T=4096, E=768, 12 heads) on 8 TRN2 NeuronCores.

Sharding: 24 (batch, head) pairs -> 3 heads per core; cores 0-3 take batch 0,
cores 4-7 take batch 1 (heads 3c..3c+2 of that batch). Each core computes
q/k/v projections for its heads, causal flash attention, and a partial output
projection (row-slice of W_proj). Host sums the 4 partial projections per
batch and adds b_proj.

Key structure (v2):
  - Per query-block j the kernel runs A(j) qkv-projection, B(j) attention,
    C(j) output projection, so ScalarE's exp() stream starts early and stays
    saturated while TensorE runs the next block's projections.
  - All matmuls run in bf16 (fp8 was tried and rejected: attention output
    magnitude is ~1/sqrt(n_eff), so P/V quantization noise passes to the
    output at full per-element strength -- fp8 P or V alone costs 2-8% rel
    error vs the 2% budget).
  - Scores are computed TRANSPOSED: S^T[tk, tq] = (k @ q^T) in bf16 with the
    D=64 contraction placed on alternating PE row-halves (tile_position
    (0,0)/(64,0)) so consecutive score matmuls run concurrently in the
    128x128 array (the halves are fed from duplicated q/k tiles).
  - exp() is evaluated in chunks of 3 key blocks from PSUM with scale=1/8
    and bias=-2 (the softmax ratio is invariant to the shared e^-2 factor;
    the bias keeps bf16 exp comfortably in range). The denominator comes for
    free from ones-columns appended to V (extra P@V output rows = column sums
    of P^T), divided out via a fast approximate reciprocal.
"""

import numpy as np
import ml_dtypes

import concourse.bass as bass
from concourse import bacc
import concourse.mybir as mybir
import concourse.tile as tile
from concourse.bass import ts
from concourse.bass_utils import run_bass_kernel_spmd

BF16 = mybir.dt.bfloat16
F32 = mybir.dt.float32
bf16 = ml_dtypes.bfloat16

B, T, E, NH = 2, 4096, 768, 12
D = E // NH            # 64 head dim
HPC = 3                # heads per core
KE = E // 128          # 6 contraction tiles over E
TQ = 512               # query-block (moving free dim)
NJ = T // TQ           # 8 query blocks
TK = 128               # key-block (scores partition dim)
NTK = T // TK          # 32 key blocks
CH = 3                 # key blocks per exp() chunk (3 PSUM banks per buffer)
N_CORES = 8
EXP_BIAS = -2.0        # exp(s/8 - 2): shared factor cancels in the softmax
import os
ALT_HALF = os.environ.get("K_ALT_HALF", "1") == "1"    # debug bisect flags
S_RESTRICT = os.environ.get("K_S_RESTRICT", "1") == "1"
DUP_SP = os.environ.get("K_DUP_SP", "0") == "1"      # dup DMAs via SP queue
PLAIN_RECIP = os.environ.get("K_PLAIN_RECIP", "1") == "1"
CONST_SP = os.environ.get("K_CONST_SP", "0") == "1"  # const loads via SP
PV_RESTRICT = os.environ.get("K_PV_RESTRICT", "1") == "1"


def _build_nc(reps=1):
    nc = bacc.Bacc()
    xT = nc.declare_dram_parameter("xT", [E, T], BF16, isOutput=False)
    wq01 = nc.declare_dram_parameter("wq01", [E, 2 * D], BF16, isOutput=False)
    wk01 = nc.declare_dram_parameter("wk01", [E, 2 * D], BF16, isOutput=False)
    wqk2 = nc.declare_dram_parameter("wqk2", [E, 2 * D], BF16, isOutput=False)
    wv = nc.declare_dram_parameter("wv", [E, HPC * D], BF16, isOutput=False)
    wp01 = nc.declare_dram_parameter("wp01", [2 * D, E], BF16, isOutput=False)
    wp2 = nc.declare_dram_parameter("wp2", [D, E], BF16, isOutput=False)
    bqk = nc.declare_dram_parameter("bqk", [2 * D, 3], F32, isOutput=False)
    bv = nc.declare_dram_parameter("bv", [1, HPC * D], F32, isOutput=False)
    msk = nc.declare_dram_parameter("msk", [TK, TK], BF16, isOutput=False)
    outT = nc.declare_dram_parameter("outT", [E, T], BF16, isOutput=True)

    add = mybir.AluOpType.add
    scale = 1.0 / np.sqrt(D)

    with tile.TileContext(nc) as tc:
        with (
            tc.tile_pool(name="const", bufs=1) as const,
            tc.tile_pool(name="ptp", bufs=3) as ptp,
            tc.tile_pool(name="ytp", bufs=2) as ytp,
            tc.tile_pool(name="yfp", bufs=2) as yfp,
            tc.tile_pool(name="outp", bufs=3) as outp,
            tc.tile_pool(name="ps_s", bufs=2, space="PSUM") as ps_s,
            tc.tile_pool(name="ps_y", bufs=1, space="PSUM") as ps_y,
            tc.tile_pool(name="ps_a", bufs=1, space="PSUM") as ps_a,
        ):
            # ---------------- constants / activations load ----------------
            # Weights/biases/masks load as single rearranged DMAs on the
            # Activation queue (idle until the first exp), concurrently with
            # the x loads on the SP queue (per-DMA sequencer dispatch is
            # ~0.6us, so queue placement matters).
            wq_sb = const.tile([128, KE, 2 * D], BF16, tag="wq")
            wk_sb = const.tile([128, KE, 2 * D], BF16, tag="wk")
            wqk2_sb = const.tile([128, KE, 2 * D], BF16, tag="wqk2")
            wv_sb = const.tile([128, KE, HPC * D], BF16, tag="wv")
            cdma = nc.sync if CONST_SP else nc.scalar
            for wsb, wdr in ((wq_sb, wq01), (wk_sb, wk01),
                             (wqk2_sb, wqk2), (wv_sb, wv)):
                cdma.dma_start(
                    out=wsb[:, :, :],
                    in_=wdr[:, :].rearrange("(ke p) m -> p ke m", p=128),
                )
            bqk_sb = const.tile([2 * D, 3], F32, tag="bqk")
            cdma.dma_start(out=bqk_sb[:, :], in_=bqk[:, :])
            bq01_sb = bqk_sb[:, 0:1]
            bk01_sb = bqk_sb[:, 1:2]
            bqk2_sb = bqk_sb[:, 2:3]
            msk_sb = const.tile([TK, TK], BF16, tag="msk")
            cdma.dma_start(out=msk_sb[:, :], in_=msk[:, :])
            wp01_sb = const.tile([2 * D, KE, 128], BF16, tag="wp01")
            cdma.dma_start(
                out=wp01_sb[:, :, :],
                in_=wp01[:, :].rearrange("d (ke p) -> d ke p", ke=KE),
            )
            wp2_sb = const.tile([D, KE, 128], BF16, tag="wp2")
            cdma.dma_start(
                out=wp2_sb[:, :, :],
                in_=wp2[:, :].rearrange("d (ke p) -> d ke p", ke=KE),
            )
            bv_sb = const.tile([128, HPC * D], F32, tag="bv")
            nc.gpsimd.dma_start(out=bv_sb[:, :], in_=bv[:, :].to_broadcast((128, HPC * D)))
            # x on the SP queue: the j<2 slices first so phase A(0)/A(1)
            # start early, then the rest as one transfer per ke chunk.
            x_sb = const.tile([128, KE, T], BF16, tag="x")
            for ke in range(KE):
                nc.sync.dma_start(out=x_sb[:, ke, 0:2 * TQ],
                                  in_=xT[ke * 128:(ke + 1) * 128, 0:2 * TQ])
            for ke in range(KE):
                nc.sync.dma_start(out=x_sb[:, ke, 2 * TQ:T],
                                  in_=xT[ke * 128:(ke + 1) * 128, 2 * TQ:T])

            # q^T/k^T tiles. Heads 0,1 stacked on partition halves; the "b"
            # tiles hold the same data with the halves swapped so consecutive
            # score matmuls can target opposite PE row-halves. Head 2 lives
            # in both halves of its own tiles.
            qT01 = const.tile([2 * D, T], BF16, tag="qT01")
            kT01 = const.tile([2 * D, T], BF16, tag="kT01")
            qT01b = const.tile([2 * D, T], BF16, tag="qT01b")
            kT01b = const.tile([2 * D, T], BF16, tag="kT01b")
            q2ab = const.tile([2 * D, T], BF16, tag="q2ab")
            k2ab = const.tile([2 * D, T], BF16, tag="k2ab")
            # v tiles with 64 appended ones-columns: the P@V matmul then emits
            # rows 0-63 = y^T and rows 64-127 = replicated column-sums of P^T
            # (the softmax denominator), so no cross-partition broadcast is
            # ever needed for the 1/l divide.
            vext = const.tile([128, HPC, NTK, 2 * D], BF16, tag="vext")
            nc.vector.memset(vext[:, :, :, D:], 1.0)

            # "Touch" DMA-loaded constants with single-input DVE copies so the
            # DMA sync-waits attach here: 2-input DVE ops (TensorTensor) only
            # have ONE sync-wait slot in the ISA encoding, and they would
            # otherwise need waits on both their PE input and these DMAs.
            scf = const.tile([128, HPC * D], F32, tag="scf")
            scb = const.tile([TK, TK], BF16, tag="scb")
            ebias = const.tile([128, 1], F32, tag="ebias")
            nc.vector.memset(ebias, EXP_BIAS)
            nc.vector.tensor_copy(out=scf[0:2 * D, 0:3], in_=bqk_sb[:, :])
            nc.vector.tensor_copy(out=scf[:, :], in_=bv_sb[:, :])
            nc.vector.tensor_copy(out=scb[:, :], in_=msk_sb[:, :])

            # (klhs, qrhs) tile per (head, PE row-half)
            def qk_pick(h, half):
                if h == 0:
                    return ((kT01, qT01) if half == 0 else (kT01b, qT01b))
                if h == 1:
                    return ((kT01b, qT01b) if half == 0 else (kT01, qT01))
                return (k2ab, q2ab)

            # ---- phase A/C work, chopped into filler pieces ----
            # Each piece is a few PE instructions plus one PSUM->SBUF drain.
            # Pieces are interleaved into phase B's chunk stream so the PE
            # executes them while ScalarE works through its exp() backlog --
            # emitting them as contiguous blocks between query-blocks would
            # starve ScalarE (the binding engine) for ~10us per block.
            def acc_ps_a():
                acc_t = ps_a.tile([128, TQ], F32, tag="acc")
                return acc_t

            def acc_ps_s():
                # Borrow a scores buffer (only legal while phase B is not
                # contending for it: A(0) before B(0), C(NJ-1) after the
                # last exp); bufs=2 gives pipelining that ps_a can't.
                acc_t = ps_s.tile([128, CH, TQ], F32, tag="s")
                return acc_t[:, 0, :]

            dup_eng = (lambda: nc.sync) if DUP_SP else (lambda: nc.gpsimd)

            def proj_piece(j, wsb, bsb, kind, acc=acc_ps_a):
                def go():
                    pps = acc()
                    for ke in range(KE):
                        nc.tensor.matmul(
                            pps,
                            wsb[:, ke, :],
                            x_sb[:, ke, ts(j, TQ)],
                            start=(ke == 0), stop=(ke == KE - 1),
                        )
                    if kind == "q01" or kind == "k01":
                        prim, dup = (qT01, qT01b) if kind == "q01" else (kT01, kT01b)
                        nc.vector.tensor_tensor(
                            out=prim[:, ts(j, TQ)], in0=pps,
                            in1=bsb.to_broadcast((2 * D, TQ)), op=add,
                        )
                        dup_eng().dma_start(out=dup[0:D, ts(j, TQ)],
                                            in_=prim[D:2 * D, ts(j, TQ)])
                        dup_eng().dma_start(out=dup[D:2 * D, ts(j, TQ)],
                                            in_=prim[0:D, ts(j, TQ)])
                    else:
                        # rows 0:64 = q2, 64:128 = k2; keep each on its
                        # producing partition half, duplicate the other half.
                        nc.vector.tensor_tensor(
                            out=q2ab[0:D, ts(j, TQ)], in0=pps[0:D, :],
                            in1=bsb[0:D, :].to_broadcast((D, TQ)), op=add,
                        )
                        nc.vector.tensor_tensor(
                            out=k2ab[D:2 * D, ts(j, TQ)], in0=pps[D:2 * D, :],
                            in1=bsb[D:2 * D, :].to_broadcast((D, TQ)), op=add,
                        )
                        dup_eng().dma_start(out=q2ab[D:2 * D, ts(j, TQ)],
                                            in_=q2ab[0:D, ts(j, TQ)])
                        dup_eng().dma_start(out=k2ab[0:D, ts(j, TQ)],
                                            in_=k2ab[D:2 * D, ts(j, TQ)])
                return go

            def v_piece(i, acc=acc_ps_a):
                def go():
                    vps = acc()[:, 0:HPC * D]
                    for ke in range(KE):
                        nc.tensor.matmul(
                            vps,
                            x_sb[:, ke, ts(i, TK)],
                            wv_sb[:, ke, :],
                            start=(ke == 0), stop=(ke == KE - 1),
                        )
                    nc.vector.tensor_tensor(
                        out=vext[:, :, i, 0:D],
                        in0=vps.rearrange("p (h d) -> p h d", h=HPC),
                        in1=bv_sb.rearrange("p (h d) -> p h d", h=HPC),
                        op=add,
                    )
                return go

            def a_pieces(j, acc=acc_ps_a):
                ps = [proj_piece(j, wq_sb, bq01_sb, "q01", acc),
                      proj_piece(j, wk_sb, bk01_sb, "k01", acc),
                      proj_piece(j, wqk2_sb, bqk2_sb, "qk2", acc)]
                ps += [v_piece(i, acc) for i in range(4 * j, 4 * j + 4)]
                return ps

            def c_piece(j, e, yt01, yt2, acc=acc_ps_a, cols=(0, TQ)):
                def go():
                    l0, l1 = cols
                    ops = acc()[:, 0:l1 - l0]
                    nc.tensor.matmul(ops, wp01_sb[:, e, :], yt01[:, l0:l1],
                                     start=True, stop=False)
                    nc.tensor.matmul(ops, wp2_sb[:, e, :], yt2[:, l0:l1],
                                     start=False, stop=True)
                    osb = outp.tile([128, TQ], BF16, tag="o")
                    nc.vector.tensor_copy(out=osb[:, 0:l1 - l0], in_=ops)
                    eng = nc.sync if j == NJ - 1 else nc.gpsimd
                    eng.dma_start(out=outT[ts(e, 128), j * TQ + l0:j * TQ + l1],
                                  in_=osb[:, 0:l1 - l0])
                return go

            for _rep in range(reps):
              fillers = []  # (due_j, fn): must run before B(due_j) starts
              for p in a_pieces(0, acc_ps_s):  # A(0) must fully precede B(0)
                  p()
              for p in a_pieces(1):
                  fillers.append((1, p))
              for j in range(NJ):
                if j + 2 < NJ:
                    fillers.extend((j + 2, p) for p in a_pieces(j + 2))
                # ---------------- phase B(j): attention ----------------
                ntk = 4 * (j + 1)  # causal: key blocks 0..4j+3
                yt01 = ytp.tile([2 * D, TQ], BF16, tag="yt01")
                yt2 = ytp.tile([D, TQ], BF16, tag="yt2")
                for h in range(HPC):
                    yps = ps_y.tile([128, TQ], F32, tag="y")
                    pvq = []  # P@V args, emitted one chunk late so the PE
                    # reaches the first P@V of a head after the previous
                    # head's divide chain has freed yps (ps_y has 1 buffer)
                    for b0 in range(0, ntk, CH):
                        bs = min(CH, ntk - b0)
                        sps = ps_s.tile([128, CH, TQ], F32, tag="s")
                        pt = ptp.tile([128, CH, TQ], BF16, tag="pt")
                        for bi in range(bs):
                            i = b0 + bi
                            half = (i % 2 if j > 0 else 0) if ALT_HALF else 0
                            klhs, qrhs = qk_pick(h, half)
                            off = D * half
                            r = i - 4 * j
                            c0 = (128 * r if r > 0 else 0) if S_RESTRICT else 0
                            nc.tensor.matmul(
                                sps[:, bi, c0:TQ],
                                klhs[off:off + D, ts(i, TK)],
                                qrhs[off:off + D, j * TQ + c0:(j + 1) * TQ],
                                start=True, stop=True,
                                **({"tile_position": (off, 0)} if ALT_HALF else {}),
                            )
                        # blocks left of the diagonal exp as one call; the
                        # diagonal suffix blocks get column-restricted calls
                        # (their left columns are never written or read)
                        n_pre = sum(1 for bi in range(bs) if b0 + bi - 4 * j <= 0) if S_RESTRICT else bs
                        if n_pre:
                            nc.scalar.activation(
                                out=pt[:, 0:n_pre, :], in_=sps[:, 0:n_pre, :],
                                func=mybir.ActivationFunctionType.Exp,
                                scale=float(scale), bias=ebias[:, :],
                            )
                        for bi in range(n_pre, bs):
                            c0 = TK * (b0 + bi - 4 * j)
                            nc.scalar.activation(
                                out=pt[:, bi, c0:TQ], in_=sps[:, bi, c0:TQ],
                                func=mybir.ActivationFunctionType.Exp,
                                scale=float(scale), bias=ebias[:, :],
                            )
                        for bi in range(bs):
                            r = b0 + bi - 4 * j
                            if r >= 0:  # diagonal block: mask the 128-wide
                                c0 = TK * r  # strip holding the diagonal
                                nc.vector.tensor_mul(
                                    pt[:, bi, c0:c0 + TK], pt[:, bi, c0:c0 + TK],
                                    msk_sb[:, :],
                                )
                        for (i, c0, ptt, bi) in pvq:
                            nc.tensor.matmul(
                                yps[:, c0:TQ],
                                vext[:, h, i, :],
                                ptt[:, bi, c0:TQ],
                                start=(i == 0), stop=(i == ntk - 1),
                            )
                        pvq = []
                        for bi in range(bs):
                            i = b0 + bi
                            r = i - 4 * j
                            pvq.append((i, (128 * r if r > 0 and PV_RESTRICT else 0), pt, bi))
                        if fillers:  # one A/C piece per exp chunk
                            fillers.pop(0)[1]()
                    for (i, c0, ptt, bi) in pvq:
                        nc.tensor.matmul(
                            yps[:, c0:TQ],
                            vext[:, h, i, :],
                            ptt[:, bi, c0:TQ],
                            start=(i == 0), stop=(i == ntk - 1),
                        )
                    lr = yfp.tile([D, TQ], F32, tag="lr")
                    if PLAIN_RECIP:
                        # NOTE: reciprocal_approx_fast (custom DVE op) returns
                        # garbage through this PJRT execution path -- verified
                        # on hardware 2026-08-08; keep the plain reciprocal.
                        nc.vector.reciprocal(out=lr, in_=yps[D:2 * D, :])
                    else:
                        nc.vector.reciprocal_approx_fast(out=lr, in_=yps[D:2 * D, :])
                    dst = yt01[h * D:(h + 1) * D, :] if h < 2 else yt2
                    nc.vector.tensor_mul(out=dst, in0=yps[0:D, :], in1=lr)
                # A(j+1) must complete before B(j+1); flush up to there.
                while any(due <= j + 1 for due, _ in fillers):
                    fillers.pop(0)[1]()
                # C(j) pieces execute during later chunk streams (C(NJ-1)
                # lands after the last exp, where the ps_s banks are free).
                if j == NJ - 1:
                    # tail: half-width pieces on the freed scores banks
                    # pipeline ~2x deeper than full-width ones would
                    fillers.extend(
                        (NJ, c_piece(j, e, yt01, yt2, acc_ps_s, (c, c + TQ // 2)))
                        for e in range(KE) for c in (0, TQ // 2))
                else:
                    fillers.extend(
                        (NJ, c_piece(j, e, yt01, yt2)) for e in range(KE))
              while fillers:
                  fillers.pop(0)[1]()
    nc.compile()
    return nc


_nc_cache = {}


def _get_nc(reps=1):
    if reps not in _nc_cache:
        _nc_cache[reps] = _build_nc(reps)
    return _nc_cache[reps]


def _make_masks():
    p = np.arange(TK)[:, None]
    c = np.arange(TK)[None, :]
    return (p <= c).astype(bf16)


def _prep_in_maps(inputs):
    x = np.asarray(inputs["x"], np.float32)
    Wa = np.asarray(inputs["W_attn"], np.float32)
    ba = np.asarray(inputs["b_attn"], np.float32)
    Wp = np.asarray(inputs["W_proj"], np.float32)
    msk = _make_masks()
    in_maps = []
    for c in range(N_CORES):
        b = c // 4
        h0 = (c % 4) * HPC * D  # column offset of this core's heads
        in_maps.append({
            "xT": np.ascontiguousarray(x[b].T).astype(bf16),
            "wq01": np.ascontiguousarray(Wa[:, h0:h0 + 2 * D]).astype(bf16),
            "wk01": np.ascontiguousarray(Wa[:, E + h0:E + h0 + 2 * D]).astype(bf16),
            "wqk2": np.ascontiguousarray(np.concatenate(
                [Wa[:, h0 + 2 * D:h0 + 3 * D],
                 Wa[:, E + h0 + 2 * D:E + h0 + 3 * D]], axis=1)).astype(bf16),
            "wv": np.ascontiguousarray(Wa[:, 2 * E + h0:2 * E + h0 + HPC * D]).astype(bf16),
            "wp01": np.ascontiguousarray(Wp[h0:h0 + 2 * D, :]).astype(bf16),
            "wp2": np.ascontiguousarray(Wp[h0 + 2 * D:h0 + HPC * D, :]).astype(bf16),
            "bqk": np.stack(
                [ba[h0:h0 + 2 * D],
                 ba[E + h0:E + h0 + 2 * D],
                 np.concatenate([ba[h0 + 2 * D:h0 + 3 * D],
                                 ba[E + h0 + 2 * D:E + h0 + 3 * D]])],
                axis=1).astype(np.float32),
            "bv": ba[2 * E + h0:2 * E + h0 + HPC * D].reshape(1, HPC * D).astype(np.float32),
            "msk": msk,
        })
    return in_maps


def _run(inputs, trace=False):
    nc = _get_nc()
    in_maps = _prep_in_maps(inputs)
    res = run_bass_kernel_spmd(nc, in_maps, core_ids=list(range(N_CORES)), trace=trace)
    bp = np.asarray(inputs["b_proj"], np.float32)
    y = np.empty((B, T, E), np.float32)
    for b in range(B):
        s = res.results[4 * b]["outT"].astype(np.float32)
        for cc in range(4 * b + 1, 4 * b + 4):
            s = s + res.results[cc]["outT"].astype(np.float32)
        y[b] = s.T
    y += bp
    return y, res


def kernel(**inputs):
    return _run(inputs)[0]
